# revision 1
# baseline (speedup 1.0000x reference)
"""Trainium2 Bass kernel for nn_AttentionBlock (pre-LN MHA with ALiBi +
pre-LN SwiGLU FFN), distributed over 8 NeuronCores.

Sharding: core = (batch, head-half). Each core computes LN1 + QKV +
attention for 8 of 16 heads over its batch's full 2048 rows, a partial
O-projection (its 512 of 1024 contraction dims), then pairwise
on-device ReduceScatters combine the partials and each core finishes
LN2 + SwiGLU FFN for half of its batch's rows.

ALiBi + causal handling: the additive alibi term s_hat*(c-r) is folded
into the score matmul via 4 augmentation rows (a c_lo/c_hi/r_lo/r_hi
split keeps every bf16 product exact); scores are computed transposed
[k, q], exponentiated without max subtraction (scores are bounded), the
causal diagonal is masked post-exp with affine_select, and the softmax
denominator comes from a ones-column appended to V.

kernel(**inputs) takes the full unsharded inputs of
reference.setup_inputs() and returns the full (4, 2048, 1024) output.
"""

import os
from contextlib import ExitStack

import numpy as np
import ml_dtypes

import concourse.bass as bass
import concourse.bacc as bacc
import concourse.mybir as mybir
import concourse.tile as tile
from concourse.masks import make_identity
from concourse.bass_utils import run_bass_kernel_spmd

BFNP = ml_dtypes.bfloat16
bf = lambda a: np.asarray(a).astype(BFNP)
f32 = lambda a: np.asarray(a, np.float32)

FP32 = mybir.dt.float32
BF16 = mybir.dt.bfloat16
AF = mybir.ActivationFunctionType

S = 2048
N_CORES = 8          # rows per batch
D = 1024          # model dim
NH = 8            # heads per core
DH = 64           # head dim
DV = NH * DH      # 512, per-core qkv dim
ROWS_OUT = 1024   # rows per core after ReduceScatter
NKT = S // 128    # 16 k-tiles
PAIRS = [[0, 1], [2, 3], [4, 5], [6, 7]]


def _build_kernel(nc):
    x = nc.dram_tensor("x", [S, D], FP32, kind="ExternalInput").ap()
    x_my = nc.dram_tensor("x_my", [ROWS_OUT, D], FP32, kind="ExternalInput").ap()
    wqT = nc.dram_tensor("wqT", [D, DV], BF16, kind="ExternalInput").ap()
    wkT = nc.dram_tensor("wkT", [D, DV], BF16, kind="ExternalInput").ap()
    wvT = nc.dram_tensor("wvT", [D, DV], BF16, kind="ExternalInput").ap()
    woT = nc.dram_tensor("woT", [DV, D], BF16, kind="ExternalInput").ap()
    w1T = nc.dram_tensor("w1T", [D, D], BF16, kind="ExternalInput").ap()
    w3T = nc.dram_tensor("w3T", [D, D], BF16, kind="ExternalInput").ap()
    w2T = nc.dram_tensor("w2T", [D, D], BF16, kind="ExternalInput").ap()
    qkvbias = nc.dram_tensor("qkvbias", [3, DV], FP32, kind="ExternalInput").ap()
    b13 = nc.dram_tensor("b13", [2, D], FP32, kind="ExternalInput").ap()
    qext = nc.dram_tensor("qext", [NH, 4, S], BF16, kind="ExternalInput").ap()
    kext = nc.dram_tensor("kext", [NH, 4, S], BF16, kind="ExternalInput").ap()

    y = nc.dram_tensor("y", [ROWS_OUT, D], FP32, kind="ExternalOutput").ap()
    with tile.TileContext(nc) as tc, ExitStack() as ctx:
        singles = ctx.enter_context(tc.tile_pool(name="singles", bufs=1))
        dram = ctx.enter_context(tc.tile_pool(name="dram", bufs=1, space="DRAM"))

        ident = singles.tile([128, 128], BF16)
        make_identity(nc, ident)

        eps = singles.tile([128, 1], FP32)
        nc.vector.memset(eps, 1e-5)

        # per-partition biases for Q/K evac ([128, 4] : dout-in-chunk x chunk)
        qb_sb = singles.tile([128, 4], FP32)
        kb_sb = singles.tile([128, 4], FP32)
        nc.sync.dma_start(out=qb_sb, in_=qkvbias[0].rearrange("(c p) -> p c", p=128))
        nc.sync.dma_start(out=kb_sb, in_=qkvbias[1].rearrange("(c p) -> p c", p=128))
        # V bias broadcast along partitions [128, 512]
        vb_sb = singles.tile([128, DV], FP32)
        nc.sync.dma_start(
            out=vb_sb,
            in_=bass.AP(tensor=qkvbias.tensor, offset=2 * DV, ap=[[0, 128], [1, DV]]),
        )
        b1_sb = singles.tile([128, 8], FP32)
        b3_sb = singles.tile([128, 8], FP32)
        nc.sync.dma_start(out=b1_sb, in_=b13[0].rearrange("(c p) -> p c", p=128))
        nc.sync.dma_start(out=b3_sb, in_=b13[1].rearrange("(c p) -> p c", p=128))

        # big tensors for phases 1-4 (one pool, closed after the O-proj)
        p_big_cm = tc.tile_pool(name="p_big", bufs=1)
        p_big = p_big_cm.__enter__()

        hTg = [p_big.tile([128, 8, 512], BF16, name=f"hT{g}", tag=f"hT{g}")
               for g in range(4)]                    # h^T row-groups of 512
        qaug = [p_big.tile([68, S], BF16, name=f"qaug{h}", tag=f"qaug{h}") for h in range(NH)]
        kaug = [p_big.tile([68, S], BF16, name=f"kaug{h}", tag=f"kaug{h}") for h in range(NH)]
        vsb = p_big.tile([128, NKT, NH * 65], BF16)  # v rows + ones col per head
        oT = p_big.tile([128, 4, S], BF16)
        o_rm = p_big.tile([128, NKT, DV], BF16)      # o row-major staging

        # ---------------- Phases 1+2 fused: LN1 + h^T + QKV, row-group interleaved ----------------
        with tc.tile_pool(name="ln1", bufs=4) as ln1p, \
             tc.tile_pool(name="xgp", bufs=3) as xgp, \
             tc.tile_pool(name="ptr1", bufs=4, space="PSUM") as ptr1, \
             tc.tile_pool(name="qkvw", bufs=1) as qkvw, \
             tc.tile_pool(name="pmm2", bufs=3, space="PSUM") as pmm2:
            wq_sb = qkvw.tile([128, 8, DV], BF16, tag="wq")
            wk_sb = qkvw.tile([128, 8, DV], BF16, tag="wk")
            wv_sb = qkvw.tile([128, 8, DV], BF16, tag="wv")
            nc.gpsimd.memset(vsb, 1.0)

            for g in range(4):
                xgs = []
                for gh in range(2):
                    xg = xgp.tile([128, 2, D], FP32, tag="xg",
                                  name=f"xg{g}_{gh}")
                    nc.sync.dma_start(
                        out=xg,
                        in_=x[g * 512 + gh * 256:g * 512 + gh * 256 + 256, :]
                            .rearrange("(a p) d -> p a d", p=128))
                    xgs.append(xg)
                for rt in range(4 * g, 4 * g + 4):
                    xt = xgs[(rt % 4) // 2][:, rt % 2, :]
                    stats = ln1p.tile([128, 2, 6], FP32, tag="stats")
                    xr = xt.rearrange("p (s f) -> p s f", s=2)
                    nc.vector.bn_stats(out=stats[:, 0, :], in_=xr[:, 0, :])
                    nc.vector.bn_stats(out=stats[:, 1, :], in_=xr[:, 1, :])
                    mv = ln1p.tile([128, 2], FP32, tag="mv")
                    nc.vector.bn_aggr(out=mv, in_=stats)
                    rstd = ln1p.tile([128, 1], FP32, tag="rstd")
                    nc.scalar.activation(out=rstd, in_=mv[:, 1:2], func=AF.Sqrt,
                                         bias=eps)
                    nc.vector.reciprocal(out=rstd, in_=rstd)
                    hrow = ln1p.tile([128, D], BF16, tag="hrow")
                    nc.vector.tensor_scalar(
                        out=hrow, in0=xt, scalar1=mv[:, 0:1], scalar2=rstd,
                        op0=mybir.AluOpType.subtract, op1=mybir.AluOpType.mult,
                    )
                    for c in range(8):
                        pt = ptr1.tile([128, 128], BF16, tag="pt")
                        nc.tensor.transpose(pt, hrow[:, c * 128:(c + 1) * 128], ident)
                        nc.scalar.copy(
                            out=hTg[rt // 4][:, c, (rt % 4) * 128:(rt % 4) * 128 + 128],
                            in_=pt)

                if g == 0:
                    nc.gpsimd.dma_start(out=wq_sb, in_=wqT.rearrange("(c p) n -> p c n", p=128))
                    nc.gpsimd.dma_start(out=wk_sb, in_=wkT.rearrange("(c p) n -> p c n", p=128))
                    nc.gpsimd.dma_start(out=wv_sb, in_=wvT.rearrange("(c p) n -> p c n", p=128))
                    for h in range(NH):
                        nc.gpsimd.dma_start(out=qaug[h][64:68, :], in_=qext[h])
                        nc.gpsimd.dma_start(out=kaug[h][64:68, :], in_=kext[h])
                # Q/K for this row group
                for (w_sb, aug, bias, scale) in (
                    (wq_sb, qaug, qb_sb, 0.125),
                    (wk_sb, kaug, kb_sb, 1.0),
                ):
                    for m in range(4):
                        ps = pmm2.tile([128, 512], FP32, tag="ps")
                        for c in range(8):
                            nc.tensor.matmul(
                                ps,
                                lhsT=w_sb[:, c, m * 128:(m + 1) * 128],
                                rhs=hTg[g][:, c, :],
                                start=(c == 0), stop=(c == 7),
                            )
                        for sub in range(2):
                            nc.scalar.activation(
                                out=aug[2 * m + sub][0:64, g * 512:(g + 1) * 512],
                                in_=ps[sub * 64:(sub + 1) * 64, :],
                                func=AF.Identity,
                                bias=bias[sub * 64:(sub + 1) * 64, m:m + 1],
                                scale=scale,
                            )
                # V for this row group (row-major out)
                for kt in range(4 * g, 4 * g + 4):
                    ps = pmm2.tile([128, 512], FP32, tag="ps")
                    for c in range(8):
                        nc.tensor.matmul(
                            ps,
                            lhsT=hTg[kt // 4][:, c, (kt % 4) * 128:(kt % 4) * 128 + 128],
                            rhs=wv_sb[:, c, :],
                            start=(c == 0), stop=(c == 7),
                        )
                    out_ap = vsb[:, kt, :].rearrange("p (h e) -> p h e", h=NH)[:, :, 0:64]
                    in_ap = ps.rearrange("p (h e) -> p h e", h=NH)
                    nc.vector.scalar_tensor_tensor(
                        out=out_ap, in0=in_ap, scalar=1.0,
                        in1=vb_sb.rearrange("p (h e) -> p h e", h=NH),
                        op0=mybir.AluOpType.mult, op1=mybir.AluOpType.add,
                    )

        # ---------------- Phase 3: attention (head pairs interleaved) ----------------
        with tc.tile_pool(name="att", bufs=4) as attp, \
             tc.tile_pool(name="psc", bufs=2, space="PSUM") as psc, \
             tc.tile_pool(name="poa", bufs=4, space="PSUM") as poa:
            for hp in range(NH // 2):
                heads = (2 * hp, 2 * hp + 1)
                for qt in range(8):          # 256-wide q tiles
                    nkt = 2 * (qt + 1)
                    oacc = [poa.tile([128, 65], FP32, name=f"oa{hp}_{qt}_{ii}",
                                     tag="oacc") for ii in range(4)]
                    for g0 in range(0, nkt, 4):
                        w = min(4, nkt - g0)
                        ats = []
                        for idx, h in enumerate(heads):
                            sc = psc.tile([128, 1024], FP32, tag="sc",
                                          name=f"sc{hp}_{qt}_{g0}_{idx}")
                            for i in range(w):
                                kt = g0 + i
                                nc.tensor.matmul(
                                    sc[:, i * 256:(i + 1) * 256],
                                    lhsT=kaug[h][:, kt * 128:(kt + 1) * 128],
                                    rhs=qaug[h][:, qt * 256:(qt + 1) * 256],
                                    start=True, stop=True,
                                )
                            at = attp.tile([128, 1024], BF16, tag="at",
                                           name=f"at{hp}_{qt}_{g0}_{idx}")
                            nc.scalar.activation(
                                out=at[:, :w * 256], in_=sc[:, :w * 256], func=AF.Exp,
                            )
                            for i in range(w):
                                kt = g0 + i
                                if kt >= nkt - 2:  # diagonal tiles: mask c > r
                                    nc.gpsimd.affine_select(
                                        out=at[:, i * 256:(i + 1) * 256],
                                        in_=at[:, i * 256:(i + 1) * 256],
                                        compare_op=mybir.AluOpType.is_ge,
                                        fill=0.0,
                                        base=qt * 256 - kt * 128,
                                        channel_multiplier=-1,
                                        pattern=[[1, 256]],
                                    )
                            ats.append(at)
                        for idx, h in enumerate(heads):
                            at = ats[idx]
                            for sub in range(2):
                                for i in range(w):
                                    kt = g0 + i
                                    nc.tensor.matmul(
                                        oacc[2 * idx + sub],
                                        lhsT=at[:, i * 256 + sub * 128:
                                                i * 256 + sub * 128 + 128],
                                        rhs=vsb[:, kt, h * 65:(h + 1) * 65],
                                        start=(kt == 0), stop=(kt == nkt - 1),
                                    )
                    for idx, h in enumerate(heads):
                        for sub in range(2):
                            oa = oacc[2 * idx + sub]
                            rec = attp.tile([128, 1], FP32, tag="rec")
                            nc.vector.reciprocal(out=rec, in_=oa[:, 64:65])
                            rr = qt * 2 + sub
                            nc.vector.tensor_scalar_mul(
                                out=o_rm[:, rr, h * 64:(h + 1) * 64],
                                in0=oa[:, 0:64], scalar1=rec,
                            )

        # Phase 3.5: transpose o to feature-major
        with tc.tile_pool(name="ptr35", bufs=4, space="PSUM") as ptr35:
            for rr in range(NKT):
                for c in range(4):
                    pt = ptr35.tile([128, 128], BF16, tag="pt")
                    nc.tensor.transpose(pt, o_rm[:, rr, c * 128:(c + 1) * 128], ident)
                    nc.scalar.copy(out=oT[:, c, rr * 128:(rr + 1) * 128], in_=pt)

        # ---------------- Phase 4: O-proj + ReduceScatter ----------------
        ccin = dram.tile([S, D], BF16)
        ccout = dram.tile([ROWS_OUT, D], BF16)
        with tc.tile_pool(name="wop", bufs=1) as wop, \
             tc.tile_pool(name="oproj", bufs=3) as op, \
             tc.tile_pool(name="pmm4", bufs=3, space="PSUM") as pmm4:
            wo_sb = wop.tile([128, 4, D], BF16, tag="wo")
            nc.sync.dma_start(out=wo_sb, in_=woT.rearrange("(c p) n -> p c n", p=128))
            for rt in range(S // 128):
                row_sb = op.tile([128, D], BF16, tag="row")
                for n in range(2):
                    ps = pmm4.tile([128, 512], FP32, tag="ps")
                    for c in range(4):
                        nc.tensor.matmul(
                            ps,
                            lhsT=oT[:, c, rt * 128:(rt + 1) * 128],
                            rhs=wo_sb[:, c, n * 512:(n + 1) * 512],
                            start=(c == 0), stop=(c == 3),
                        )
                    nc.scalar.copy(out=row_sb[:, n * 512:(n + 1) * 512], in_=ps)
                nc.sync.dma_start(out=ccin[rt * 128:(rt + 1) * 128, :], in_=row_sb)
            nc.gpsimd.collective_compute(
                "ReduceScatter",
                mybir.AluOpType.add,
                ins=[ccin.opt()],
                outs=[ccout.opt()],
                replica_groups=PAIRS,
            )

        p_big_cm.__exit__(None, None, None)
        p_x2_cm = tc.tile_pool(name="p_x2", bufs=1)          # phases 5-6
        p_x2 = p_x2_cm.__enter__()
        x2_sb = p_x2.tile([128, 8, D], FP32)
        h2g = [p_x2.tile([128, 8, 512], BF16, name=f"h2T{g}", tag=f"h2T{g}")
               for g in range(2)]


        # ---------------- Phase 5: x2 + LN2 + h2^T ----------------
        with tc.tile_pool(name="ln2", bufs=3) as ln2p, \
             tc.tile_pool(name="ptr5", bufs=4, space="PSUM") as ptr5:
            for rt in range(ROWS_OUT // 128):
                xt = ln2p.tile([128, D], FP32, tag="xt")
                nc.sync.dma_start(out=xt, in_=x_my[rt * 128:(rt + 1) * 128, :])
                rs = ln2p.tile([128, D], BF16, tag="rs")
                nc.sync.dma_start(out=rs, in_=ccout[rt * 128:(rt + 1) * 128, :])
                nc.vector.tensor_add(x2_sb[:, rt, :], xt, rs)
                stats = ln2p.tile([128, 2, 6], FP32, tag="stats")
                x2r = x2_sb[:, rt, :].rearrange("p (s f) -> p s f", s=2)
                nc.vector.bn_stats(out=stats[:, 0, :], in_=x2r[:, 0, :])
                nc.vector.bn_stats(out=stats[:, 1, :], in_=x2r[:, 1, :])
                mv = ln2p.tile([128, 2], FP32, tag="mv")
                nc.vector.bn_aggr(out=mv, in_=stats)
                rstd = ln2p.tile([128, 1], FP32, tag="rstd")
                nc.scalar.activation(out=rstd, in_=mv[:, 1:2], func=AF.Sqrt, bias=eps)
                nc.vector.reciprocal(out=rstd, in_=rstd)
                hrow = ln2p.tile([128, D], BF16, tag="hrow")
                nc.vector.tensor_scalar(
                    out=hrow, in0=x2_sb[:, rt, :], scalar1=mv[:, 0:1], scalar2=rstd,
                    op0=mybir.AluOpType.subtract, op1=mybir.AluOpType.mult,
                )
                for c in range(8):
                    pt = ptr5.tile([128, 128], BF16, tag="pt")
                    nc.tensor.transpose(pt, hrow[:, c * 128:(c + 1) * 128], ident)
                    nc.scalar.copy(
                        out=h2g[rt // 4][:, c, (rt % 4) * 128:(rt % 4) * 128 + 128],
                        in_=pt)

        # ---------------- Phase 6: FFN ----------------
        with tc.tile_pool(name="ffnw", bufs=1) as ffnw, \
             tc.tile_pool(name="ffn2", bufs=3) as ffn2, \
             tc.tile_pool(name="pmm6", bufs=3, space="PSUM") as pmm6:
            w1_sb = ffnw.tile([128, 8, D], BF16, tag="w1")
            w3_sb = ffnw.tile([128, 8, D], BF16, tag="w3")
            w2_sb = ffnw.tile([128, 8, D], BF16, tag="w2")
            nc.sync.dma_start(out=w1_sb, in_=w1T.rearrange("(c p) n -> p c n", p=128))
            nc.sync.dma_start(out=w3_sb, in_=w3T.rearrange("(c p) n -> p c n", p=128))
            nc.sync.dma_start(out=w2_sb, in_=w2T.rearrange("(c p) n -> p c n", p=128))
            gs = ffnw.tile([128, 8, ROWS_OUT], BF16, tag="gs")
            for f in range(8):
                for r2 in range(2):
                    ps = pmm6.tile([128, 512], FP32, tag="ps")
                    for c in range(8):
                        nc.tensor.matmul(
                            ps,
                            lhsT=w1_sb[:, c, f * 128:(f + 1) * 128],
                            rhs=h2g[r2][:, c, :],
                            start=(c == 0), stop=(c == 7),
                        )
                    us = ffn2.tile([128, 512], BF16, tag="us")
                    nc.scalar.activation(
                        out=us, in_=ps, func=AF.Silu, bias=b1_sb[:, f:f + 1],
                    )
                    ps2 = pmm6.tile([128, 512], FP32, tag="ps")
                    for c in range(8):
                        nc.tensor.matmul(
                            ps2,
                            lhsT=w3_sb[:, c, f * 128:(f + 1) * 128],
                            rhs=h2g[r2][:, c, :],
                            start=(c == 0), stop=(c == 7),
                        )
                    ts = ffn2.tile([128, 512], BF16, tag="ts")
                    nc.vector.tensor_scalar(
                        out=ts, in0=ps2, scalar1=b3_sb[:, f:f + 1], scalar2=None,
                        op0=mybir.AluOpType.add,
                    )
                    nc.vector.tensor_mul(gs[:, f, r2 * 512:(r2 + 1) * 512], us, ts)
            for rt in range(ROWS_OUT // 128):
                ysb = ffn2.tile([128, D], FP32, tag="ysb")
                for n in range(2):
                    ps = pmm6.tile([128, 512], FP32, tag="ps")
                    for f in range(8):
                        nc.tensor.matmul(
                            ps,
                            lhsT=gs[:, f, rt * 128:(rt + 1) * 128],
                            rhs=w2_sb[:, f, n * 512:(n + 1) * 512],
                            start=(f == 0), stop=(f == 7),
                        )
                    nc.vector.tensor_add(
                        ysb[:, n * 512:(n + 1) * 512], ps,
                        x2_sb[:, rt, n * 512:(n + 1) * 512],
                    )
                nc.sync.dma_start(out=y[rt * 128:(rt + 1) * 128, :], in_=ysb)

        p_x2_cm.__exit__(None, None, None)

    return nc

H = 16
_SLOPES = (2.0 ** (-8.0 / H)) ** np.arange(1, H + 1)
_RIDX = np.arange(S, dtype=np.float64)


def _prep_core_inputs(inp, core):
    b, half = core // 2, core % 2
    hsl = slice(half * DV, (half + 1) * DV)
    g1, b1 = f32(inp["ln1_g"]), f32(inp["ln1_b"])
    g2, b2 = f32(inp["ln2_g"]), f32(inp["ln2_b"])
    wq, wk, wv, wo = (f32(inp[k]) for k in ("wq", "wk", "wv", "wo"))
    w1, w2, w3 = (f32(inp[k]) for k in ("w1", "w2", "w3"))

    qkvbias = np.stack([
        (wq[hsl] @ b1) / 8.0,
        wk[hsl] @ b1,
        wv[hsl] @ b1,
    ]).astype(np.float32)
    b13 = np.stack([w1 @ b2, w3 @ b2]).astype(np.float32)

    qext = np.zeros((NH, 4, S), BFNP)
    kext = np.zeros((NH, 4, S), BFNP)
    for j in range(NH):
        s_hat = float(bf(np.float32(_SLOPES[half * NH + j])))
        qext[j, 0] = bf(np.full(S, s_hat))
        qext[j, 1] = bf(np.full(S, 256.0 * s_hat))
        qext[j, 2] = bf(-(_RIDX % 256))
        qext[j, 3] = bf(-(_RIDX // 256))
        kext[j, 0] = bf(_RIDX % 256)
        kext[j, 1] = bf(_RIDX // 256)
        kext[j, 2] = bf(np.full(S, s_hat))
        kext[j, 3] = bf(np.full(S, 256.0 * s_hat))

    out = {
        "x": f32(inp["x"][b]),
        "x_my": f32(inp["x"][b][half * ROWS_OUT:(half + 1) * ROWS_OUT]),
        "wqT": bf((wq[hsl] * g1[None, :]).T),
        "wkT": bf((wk[hsl] * g1[None, :]).T),
        "wvT": bf((wv[hsl] * g1[None, :]).T),
        "woT": bf(wo[:, hsl].T),
        "w1T": bf((w1 * g2[None, :]).T),
        "w3T": bf((w3 * g2[None, :]).T),
        "w2T": bf(w2.T),
        "qkvbias": qkvbias,
        "b13": b13,
        "qext": qext,
        "kext": kext,
    }
    return {k: np.ascontiguousarray(v) for k, v in out.items()}


_COMPILED_NC = None
LAST_EXEC_NS = None


def _get_nc():
    global _COMPILED_NC
    if _COMPILED_NC is None:
        nc = bacc.Bacc("TRN2", target_bir_lowering=False, debug=False,
                       num_devices=N_CORES)
        _build_kernel(nc)
        nc.compile()
        _COMPILED_NC = nc
    return _COMPILED_NC


def kernel(**inputs):
    global LAST_EXEC_NS
    nc = _get_nc()
    in_maps = [_prep_core_inputs(inputs, c) for c in range(N_CORES)]
    trace = os.environ.get("KERNEL_TRACE", "0") == "1"
    res = run_bass_kernel_spmd(nc, in_maps, list(range(N_CORES)), trace=trace)
    LAST_EXEC_NS = res.exec_time_ns
    B = inputs["x"].shape[0]
    y = np.zeros((B, S, D), np.float32)
    for core in range(N_CORES):
        b, half = core // 2, core % 2
        y[b, half * ROWS_OUT:(half + 1) * ROWS_OUT] = res.results[core]["y"]
    return y



# revision 5
# speedup vs baseline: 1.0100x; 1.0100x over previous
"""Trainium2 Bass kernel for nn_AttentionBlock (pre-LN MHA with ALiBi +
pre-LN SwiGLU FFN), distributed over 8 NeuronCores.

Sharding: core = (batch, head-half). Each core computes LN1 + QKV +
attention for 8 of 16 heads over its batch's full 2048 rows, a partial
O-projection, then quarter-chunked pairwise ReduceScatters combine the
partials (overlapped with the attention tail) and each core finishes
LN2 + SwiGLU FFN for its 1024 rows.

Precision: fp8e4m3 DoubleRow matmuls for QKV, AV, and FFN mm1/mm3
(weights pre-scaled x32 to avoid fp8 subnormals); bf16 for scores
(ALiBi folded into 4 augmentation contraction rows), O-proj and FFN
mm2. Attention is processed span-by-span (512 q rows), fully
interleaved with LN1/QKV production, with the AV matmul transposed
(out = v^T @ at) so the softmax denominator is a free 65th output row
and no o-transposes are needed.
"""

import os
from contextlib import ExitStack

import numpy as np
import ml_dtypes

import concourse.bass as bass
import concourse.bacc as bacc
import concourse.mybir as mybir
import concourse.tile as tile
from concourse.masks import make_identity
from concourse.bass_utils import run_bass_kernel_spmd

BFNP = ml_dtypes.bfloat16
F8NP = ml_dtypes.float8_e4m3
bf = lambda a: np.asarray(a).astype(BFNP)
f8 = lambda a: np.asarray(a, np.float32).astype(F8NP)
f32 = lambda a: np.asarray(a, np.float32)

FP32 = mybir.dt.float32
BF16 = mybir.dt.bfloat16
FP8 = mybir.dt.float8e4
AF = mybir.ActivationFunctionType
ALU = mybir.AluOpType
DR = mybir.MatmulPerfMode.DoubleRow

S = 2048
N_CORES = 8
D = 1024          # model dim
NH = 8            # heads per core
DH = 64           # head dim
DV = NH * DH      # 512, per-core qkv dim
ROWS_OUT = 1024   # rows per core after ReduceScatter
NKT = S // 128    # 16 k-tiles
NSP = 4           # spans of 512 q rows
WS = 32.0         # fp8 weight pre-scale
PAIRS = [[0, 1], [2, 3], [4, 5], [6, 7]]


def _build_kernel(nc):
    x = nc.dram_tensor("x", [S, D], FP32, kind="ExternalInput").ap()
    x_my = nc.dram_tensor("x_my", [ROWS_OUT, D], FP32, kind="ExternalInput").ap()
    wq8 = nc.dram_tensor("wq8", [D, DV], FP8, kind="ExternalInput").ap()
    wk8 = nc.dram_tensor("wk8", [D, DV], FP8, kind="ExternalInput").ap()
    wv8 = nc.dram_tensor("wv8", [D, DV], FP8, kind="ExternalInput").ap()
    woT = nc.dram_tensor("woT", [DV, D], BF16, kind="ExternalInput").ap()
    w18 = nc.dram_tensor("w18", [D, D], FP8, kind="ExternalInput").ap()
    w38 = nc.dram_tensor("w38", [D, D], FP8, kind="ExternalInput").ap()
    w2T = nc.dram_tensor("w2T", [D, D], BF16, kind="ExternalInput").ap()
    qkvbias = nc.dram_tensor("qkvbias", [3, DV], FP32, kind="ExternalInput").ap()
    b13 = nc.dram_tensor("b13", [2, D], FP32, kind="ExternalInput").ap()
    qext = nc.dram_tensor("qext", [NH, 4, S], BF16, kind="ExternalInput").ap()
    kext = nc.dram_tensor("kext", [NH, 4, S], BF16, kind="ExternalInput").ap()

    y = nc.dram_tensor("y", [ROWS_OUT, D], FP32, kind="ExternalOutput").ap()

    with tile.TileContext(nc) as tc, ExitStack() as ctx:
        singles = ctx.enter_context(tc.tile_pool(name="singles", bufs=1))
        dram = ctx.enter_context(tc.tile_pool(name="dram", bufs=1, space="DRAM"))

        ident = singles.tile([128, 128], BF16)
        make_identity(nc, ident)

        # per-partition biases for Q/K evac ([128, 4] : dout-in-chunk x chunk)
        qb_sb = singles.tile([128, 4], FP32)
        kb_sb = singles.tile([128, 4], FP32)
        nc.sync.dma_start(out=qb_sb, in_=qkvbias[0].rearrange("(c p) -> p c", p=128))
        nc.sync.dma_start(out=kb_sb, in_=qkvbias[1].rearrange("(c p) -> p c", p=128))
        # V bias broadcast along partitions [128, 512]
        vb_sb = singles.tile([128, DV], FP32)
        nc.sync.dma_start(
            out=vb_sb,
            in_=bass.AP(tensor=qkvbias.tensor, offset=2 * DV, ap=[[0, 128], [1, DV]]),
        )
        b1_sb = singles.tile([128, 8], FP32)
        b3_sb = singles.tile([128, 8], FP32)
        nc.sync.dma_start(out=b1_sb, in_=b13[0].rearrange("(c p) -> p c", p=128))
        nc.sync.dma_start(out=b3_sb, in_=b13[1].rearrange("(c p) -> p c", p=128))

        # x2 residual rows (mine) - filled with x_my early, += RS result later
        p_x2_cm = tc.tile_pool(name="p_x2", bufs=1)
        p_x2 = p_x2_cm.__enter__()
        x2_sb = p_x2.tile([128, 8, D], FP32)

        # persistent tensors for phases 1-4
        p_big_cm = tc.tile_pool(name="p_big", bufs=1)
        p_big = p_big_cm.__enter__()
        hT = p_big.tile([128, 8, S], FP8)        # LN1-out transposed, c x rows
        qaug = [p_big.tile([68, S], BF16, name=f"qaug{h}", tag=f"qaug{h}")
                for h in range(NH)]
        kaug = [p_big.tile([68, S], BF16, name=f"kaug{h}", tag=f"kaug{h}")
                for h in range(NH)]
        vsb = p_big.tile([128, NKT, NH, 72], FP8)  # v rows + ones col, pad 72
        oT = p_big.tile([128, 4, S], BF16)         # normalized o, feature-major
        wo_sb = p_big.tile([128, 4, D], BF16)

        # DRAM comm buffers: quarter q rows = [1024k + 256q, +256) for rank k
        ccin = [dram.tile([2, 256, D], BF16, name=f"ccin{q}") for q in range(4)]
        ccout = [dram.tile([256, D], BF16, name=f"ccout{q}") for q in range(4)]

        # ---------------- phases 1-3 interleaved by span ----------------
        with tc.tile_pool(name="qkvw", bufs=1) as qkvw, \
             tc.tile_pool(name="xgp", bufs=2) as xgp, \
             tc.tile_pool(name="ln1", bufs=3) as ln1p, \
             tc.tile_pool(name="attp", bufs=3) as attp, \
             tc.tile_pool(name="atd", bufs=1) as atdp, \
             tc.tile_pool(name="oprs", bufs=2) as oprs, \
             tc.tile_pool(name="pwork", bufs=2, space="PSUM") as pwork, \
             tc.tile_pool(name="pacc", bufs=3, space="PSUM") as pacc, \
             tc.tile_pool(name="ptr", bufs=1, space="PSUM") as ptrp:
            wq_sb = qkvw.tile([128, 8, DV], FP8, tag="wq")
            wk_sb = qkvw.tile([128, 8, DV], FP8, tag="wk")
            wv_sb = qkvw.tile([128, 8, DV], FP8, tag="wv")
            nc.gpsimd.dma_start(out=wq_sb, in_=wq8.rearrange("(c p) n -> p c n", p=128))
            nc.gpsimd.dma_start(out=wk_sb, in_=wk8.rearrange("(c p) n -> p c n", p=128))
            nc.gpsimd.dma_start(out=wv_sb, in_=wv8.rearrange("(c p) n -> p c n", p=128))
            nc.gpsimd.dma_start(out=wo_sb, in_=woT.rearrange("(c p) n -> p c n", p=128))
            nc.gpsimd.memset(vsb, 1.0)
            for h in range(NH):
                nc.gpsimd.dma_start(out=qaug[h][64:68, :], in_=qext[h])
                nc.gpsimd.dma_start(out=kaug[h][64:68, :], in_=kext[h])
            # x_my prefetch into x2 residual buffer
            for rt in range(8):
                nc.sync.dma_start(
                    out=x2_sb[:, rt, :], in_=x_my[rt * 128:(rt + 1) * 128, :])
            # dedicated diagonal at tiles (pre-zeroed masked regions), by
            # in-span diag pair position p and head parity
            at_diag = [[atdp.tile([128, 2, 512], FP8, name=f"atd{p}_{par}",
                                  tag=f"atd{p}_{par}")
                        for par in range(2)] for p in range(2)]
            for p in range(2):
                for par in range(2):
                    nc.vector.memset(at_diag[p][par], 0.0)

            for g in range(NSP):
                # ---- LN1 + transpose for row group g (512 rows) ----
                xgs = []
                for gh in range(2):
                    xg = xgp.tile([128, 2, D], FP32, tag="xg", name=f"xg{g}_{gh}")
                    nc.sync.dma_start(
                        out=xg,
                        in_=x[g * 512 + gh * 256:g * 512 + gh * 256 + 256, :]
                            .rearrange("(a p) d -> p a d", p=128))
                    xgs.append(xg)
                for rt in range(4 * g, 4 * g + 4):
                    xt = xgs[(rt % 4) // 2][:, rt % 2, :]
                    stats = ln1p.tile([128, 2, 6], FP32, tag="stats")
                    xr = xt.rearrange("p (s f) -> p s f", s=2)
                    nc.vector.bn_stats(out=stats[:, 0, :], in_=xr[:, 0, :])
                    nc.vector.bn_stats(out=stats[:, 1, :], in_=xr[:, 1, :])
                    mv = ln1p.tile([128, 2], FP32, tag="mv")
                    nc.vector.bn_aggr(out=mv, in_=stats)
                    # rstd = rsqrt(var + eps) via 2 Newton iterations from 1.0
                    ve = ln1p.tile([128, 3], FP32, tag="ve")
                    nc.vector.tensor_scalar(
                        out=ve[:, 0:1], in0=mv[:, 1:2], scalar1=1.0, scalar2=1e-5,
                        op0=ALU.mult, op1=ALU.add)
                    nc.vector.tensor_scalar(
                        out=ve[:, 1:2], in0=ve[:, 0:1], scalar1=-0.5, scalar2=1.5,
                        op0=ALU.mult, op1=ALU.add)
                    nc.vector.tensor_mul(ve[:, 2:3], ve[:, 1:2], ve[:, 1:2])
                    nc.vector.tensor_mul(ve[:, 2:3], ve[:, 2:3], ve[:, 0:1])
                    nc.vector.tensor_scalar(
                        out=ve[:, 2:3], in0=ve[:, 2:3], scalar1=-0.5, scalar2=1.5,
                        op0=ALU.mult, op1=ALU.add)
                    rstd = ln1p.tile([128, 1], FP32, tag="rstd")
                    nc.vector.tensor_mul(rstd, ve[:, 1:2], ve[:, 2:3])
                    hrow = ln1p.tile([128, D], BF16, tag="hrow")
                    nc.vector.tensor_scalar(
                        out=hrow, in0=xt, scalar1=mv[:, 0:1], scalar2=rstd,
                        op0=ALU.subtract, op1=ALU.mult)
                    pt = ptrp.tile([128, 8, 128], BF16, tag="pt")
                    for c in range(8):
                        nc.tensor.transpose(pt[:, c, :], hrow[:, c * 128:(c + 1) * 128],
                                            ident)
                    nc.vector.tensor_copy(hT[:, :, rt * 128:(rt + 1) * 128], pt)

                # ---- QKV for row group g (fp8 DoubleRow) ----
                for (w_sb, aug, bias, scale) in (
                    (wq_sb, qaug, qb_sb, 0.125 / WS),
                    (wk_sb, kaug, kb_sb, 1.0 / WS),
                ):
                    for m in range(4):
                        ps = pwork.tile([128, 2, 512], FP32, tag="w")
                        for c in range(4):
                            nc.tensor.matmul(
                                ps[:, 0, :],
                                lhsT=w_sb[:, 2 * c:2 * c + 2, m * 128:(m + 1) * 128],
                                rhs=hT[:, 2 * c:2 * c + 2, g * 512:(g + 1) * 512],
                                start=(c == 0), stop=(c == 3), perf_mode=DR)
                        for sub in range(2):
                            nc.vector.tensor_scalar(
                                out=aug[2 * m + sub][0:64, g * 512:(g + 1) * 512],
                                in0=ps[sub * 64:sub * 64 + 64, 0, :],
                                scalar1=scale,
                                scalar2=bias[sub * 64:sub * 64 + 64, m:m + 1],
                                op0=ALU.mult, op1=ALU.add)
                for kt in range(4 * g, 4 * g + 4):
                    ps = pwork.tile([128, 2, 512], FP32, tag="w")
                    for c in range(4):
                        nc.tensor.matmul(
                            ps[:, 0, :],
                            lhsT=hT[:, 2 * c:2 * c + 2, kt * 128:(kt + 1) * 128],
                            rhs=wv_sb[:, 2 * c:2 * c + 2, :],
                            start=(c == 0), stop=(c == 3), perf_mode=DR)
                    nc.vector.scalar_tensor_tensor(
                        out=vsb[:, kt, :, 0:64],
                        in0=ps[:, 0, :].rearrange("p (h e) -> p h e", h=NH),
                        scalar=1.0 / WS,
                        in1=vb_sb.rearrange("p (h e) -> p h e", h=NH),
                        op0=ALU.mult, op1=ALU.add)

                # ---- attention span g for all heads ----
                npair = 2 * (g + 1)
                for h in range(NH):
                    oacc = pacc.tile([128, 512], FP32, tag="acc",
                                     name=f"oa{g}_{h}")
                    for i in range(npair):
                        diag = i >= 2 * g
                        sc = pwork.tile([128, 2, 512], FP32, tag="w",
                                        name=f"sc{g}_{h}_{i}")
                        if not diag:
                            for j in range(2):
                                kt = 2 * i + j
                                nc.tensor.matmul(
                                    sc[:, j, :],
                                    lhsT=kaug[h][:, kt * 128:(kt + 1) * 128],
                                    rhs=qaug[h][:, g * 512:(g + 1) * 512],
                                    start=True, stop=True)
                            at = attp.tile([128, 2, 512], FP8, tag="at",
                                           name=f"at{g}_{h}_{i}")
                            nc.scalar.activation(out=at, in_=sc, func=AF.Exp)
                        else:
                            p = i - 2 * g
                            at = at_diag[p][h % 2]
                            for j in range(2):
                                kt = 2 * i + j
                                jp = kt - 4 * g    # 0..3 within diag band
                                nc.tensor.matmul(
                                    sc[:, j, jp * 128:512],
                                    lhsT=kaug[h][:, kt * 128:(kt + 1) * 128],
                                    rhs=qaug[h][:, g * 512 + jp * 128:(g + 1) * 512],
                                    start=True, stop=True)
                                nc.scalar.activation(
                                    out=at[:, j, jp * 128:512],
                                    in_=sc[:, j, jp * 128:512], func=AF.Exp)
                                nc.gpsimd.affine_select(
                                    out=at[:, j, jp * 128:jp * 128 + 128],
                                    in_=at[:, j, jp * 128:jp * 128 + 128],
                                    compare_op=ALU.is_ge, fill=0.0,
                                    base=0, channel_multiplier=-1,
                                    pattern=[[1, 128]])
                        nc.tensor.matmul(
                            oacc[0:65, :],
                            lhsT=vsb[:, 2 * i:2 * i + 2, h, 0:65],
                            rhs=at,
                            start=(i == 0), stop=(i == npair - 1), perf_mode=DR)
                    # normalize + evac into oT
                    rec = attp.tile([1, 512], BF16, tag="rec")
                    with nc.allow_low_precision(reason="softmax recip in bf16"):
                        nc.vector.reciprocal(out=rec, in_=oacc[64:65, :])
                    rb = attp.tile([64, 512], BF16, tag="rb")
                    nc.gpsimd.partition_broadcast(rb, rec, channels=64)
                    nc.vector.tensor_mul(
                        oT[(h % 2) * 64:(h % 2) * 64 + 64, h // 2,
                           g * 512:(g + 1) * 512],
                        oacc[0:64, :], rb)

                # ---- O-proj partials for span g + ccin DMA ----
                for rt4 in range(4):
                    rowslc = slice(g * 512 + rt4 * 128, g * 512 + rt4 * 128 + 128)
                    ob = oprs.tile([128, D], BF16, tag="ob")
                    for n in range(2):
                        ps = pwork.tile([128, 2, 512], FP32, tag="w")
                        for c in range(4):
                            nc.tensor.matmul(
                                ps[:, 0, :],
                                lhsT=oT[:, c, rowslc],
                                rhs=wo_sb[:, c, n * 512:(n + 1) * 512],
                                start=(c == 0), stop=(c == 3))
                        nc.vector.tensor_copy(ob[:, n * 512:(n + 1) * 512],
                                              ps[:, 0, :])
                    qq = 2 * (g % 2) + rt4 // 2
                    k = g // 2
                    u = rt4 % 2
                    nc.sync.dma_start(
                        out=ccin[qq][k][u * 128:u * 128 + 128, :], in_=ob)
                # fire quarter ReduceScatters when both spans contributed
                if g == 2:
                    for qq in range(2):
                        nc.gpsimd.collective_compute(
                            "ReduceScatter", ALU.add,
                            ins=[ccin[qq].opt()], outs=[ccout[qq].opt()],
                            replica_groups=PAIRS)
                if g == 3:
                    for qq in range(2, 4):
                        nc.gpsimd.collective_compute(
                            "ReduceScatter", ALU.add,
                            ins=[ccin[qq].opt()], outs=[ccout[qq].opt()],
                            replica_groups=PAIRS)

        p_big_cm.__exit__(None, None, None)

        # ---------------- phases 5-6 per quarter ----------------
        with tc.tile_pool(name="ffnw", bufs=1) as ffnw, \
             tc.tile_pool(name="ln2", bufs=3) as ln2p, \
             tc.tile_pool(name="ffn2", bufs=2) as ffn2, \
             tc.tile_pool(name="pacc6", bufs=3, space="PSUM") as pacc6, \
             tc.tile_pool(name="ptr5", bufs=1, space="PSUM") as ptr5:
            w1_sb = ffnw.tile([128, 8, D], FP8, tag="w1")
            w3_sb = ffnw.tile([128, 8, D], FP8, tag="w3")
            w2_sb = ffnw.tile([128, 8, D], BF16, tag="w2")
            nc.gpsimd.dma_start(out=w1_sb, in_=w18.rearrange("(c p) n -> p c n", p=128))
            nc.gpsimd.dma_start(out=w3_sb, in_=w38.rearrange("(c p) n -> p c n", p=128))
            nc.gpsimd.dma_start(out=w2_sb, in_=w2T.rearrange("(c p) n -> p c n", p=128))
            h2T = ffnw.tile([128, 8, ROWS_OUT], FP8, tag="h2T")

            for qq in range(4):
                rs = ln2p.tile([128, 2, D], BF16, tag="rs")
                nc.sync.dma_start(
                    out=rs, in_=ccout[qq].rearrange("(a p) d -> p a d", p=128))
                for u in range(2):
                    rt = 2 * qq + u
                    nc.vector.tensor_add(x2_sb[:, rt, :], rs[:, u, :],
                                         x2_sb[:, rt, :])
                    stats = ln2p.tile([128, 2, 6], FP32, tag="stats")
                    x2r = x2_sb[:, rt, :].rearrange("p (s f) -> p s f", s=2)
                    nc.vector.bn_stats(out=stats[:, 0, :], in_=x2r[:, 0, :])
                    nc.vector.bn_stats(out=stats[:, 1, :], in_=x2r[:, 1, :])
                    mv = ln2p.tile([128, 2], FP32, tag="mv")
                    nc.vector.bn_aggr(out=mv, in_=stats)
                    ve = ln2p.tile([128, 3], FP32, tag="ve")
                    nc.vector.tensor_scalar(
                        out=ve[:, 0:1], in0=mv[:, 1:2], scalar1=1.0, scalar2=1e-5,
                        op0=ALU.mult, op1=ALU.add)
                    nc.vector.tensor_scalar(
                        out=ve[:, 1:2], in0=ve[:, 0:1], scalar1=-0.5, scalar2=1.5,
                        op0=ALU.mult, op1=ALU.add)
                    nc.vector.tensor_mul(ve[:, 2:3], ve[:, 1:2], ve[:, 1:2])
                    nc.vector.tensor_mul(ve[:, 2:3], ve[:, 2:3], ve[:, 0:1])
                    nc.vector.tensor_scalar(
                        out=ve[:, 2:3], in0=ve[:, 2:3], scalar1=-0.5, scalar2=1.5,
                        op0=ALU.mult, op1=ALU.add)
                    rstd = ln2p.tile([128, 1], FP32, tag="rstd")
                    nc.vector.tensor_mul(rstd, ve[:, 1:2], ve[:, 2:3])
                    hrow = ln2p.tile([128, D], BF16, tag="hrow")
                    nc.vector.tensor_scalar(
                        out=hrow, in0=x2_sb[:, rt, :], scalar1=mv[:, 0:1],
                        scalar2=rstd, op0=ALU.subtract, op1=ALU.mult)
                    pt = ptr5.tile([128, 8, 128], BF16, tag="pt")
                    for c in range(8):
                        nc.tensor.transpose(pt[:, c, :], hrow[:, c * 128:(c + 1) * 128],
                                            ident)
                    nc.vector.tensor_copy(h2T[:, :, rt * 128:(rt + 1) * 128], pt)

                # FFN for this quarter (256 rows)
                cols = slice(qq * 256, qq * 256 + 256)
                gs = ffn2.tile([128, 8, 256], BF16, tag="gs")
                for f in range(8):
                    ps = pacc6.tile([128, 512], FP32, tag="acc6")
                    for c in range(4):
                        nc.tensor.matmul(
                            ps[:, 0:256],
                            lhsT=w1_sb[:, 2 * c:2 * c + 2, f * 128:(f + 1) * 128],
                            rhs=h2T[:, 2 * c:2 * c + 2, cols],
                            start=(c == 0), stop=(c == 3), perf_mode=DR)
                    us = ffn2.tile([128, 256], BF16, tag="us")
                    nc.scalar.activation(
                        out=us, in_=ps[:, 0:256], func=AF.Silu,
                        bias=b1_sb[:, f:f + 1], scale=1.0 / WS)
                    ps2 = pacc6.tile([128, 512], FP32, tag="acc6")
                    for c in range(4):
                        nc.tensor.matmul(
                            ps2[:, 0:256],
                            lhsT=w3_sb[:, 2 * c:2 * c + 2, f * 128:(f + 1) * 128],
                            rhs=h2T[:, 2 * c:2 * c + 2, cols],
                            start=(c == 0), stop=(c == 3), perf_mode=DR)
                    ts = ffn2.tile([128, 256], BF16, tag="ts")
                    nc.vector.tensor_scalar(
                        out=ts, in0=ps2[:, 0:256], scalar1=1.0 / WS,
                        scalar2=b3_sb[:, f:f + 1], op0=ALU.mult, op1=ALU.add)
                    nc.vector.tensor_mul(gs[:, f, :], us, ts)
                for u in range(2):
                    rt = 2 * qq + u
                    ysb = ffn2.tile([128, D], FP32, tag="ysb")
                    for n in range(2):
                        ps = pacc6.tile([128, 512], FP32, tag="acc6")
                        for f in range(8):
                            nc.tensor.matmul(
                                ps,
                                lhsT=gs[:, f, u * 128:u * 128 + 128],
                                rhs=w2_sb[:, f, n * 512:(n + 1) * 512],
                                start=(f == 0), stop=(f == 7))
                        nc.vector.tensor_add(
                            ysb[:, n * 512:(n + 1) * 512], ps,
                            x2_sb[:, rt, n * 512:(n + 1) * 512])
                    nc.sync.dma_start(out=y[rt * 128:(rt + 1) * 128, :], in_=ysb)

        p_x2_cm.__exit__(None, None, None)
    return nc


H = 16
_SLOPES = (2.0 ** (-8.0 / H)) ** np.arange(1, H + 1)
_RIDX = np.arange(S, dtype=np.float64)


def _prep_core_inputs(inp, core):
    b, half = core // 2, core % 2
    hsl = slice(half * DV, (half + 1) * DV)
    g1, b1 = f32(inp["ln1_g"]), f32(inp["ln1_b"])
    g2, b2 = f32(inp["ln2_g"]), f32(inp["ln2_b"])
    wq, wk, wv, wo = (f32(inp[k]) for k in ("wq", "wk", "wv", "wo"))
    w1, w2, w3 = (f32(inp[k]) for k in ("w1", "w2", "w3"))

    qkvbias = np.stack([
        (wq[hsl] @ b1) / 8.0,
        wk[hsl] @ b1,
        wv[hsl] @ b1,
    ]).astype(np.float32)
    b13 = np.stack([w1 @ b2, w3 @ b2]).astype(np.float32)

    qext = np.zeros((NH, 4, S), BFNP)
    kext = np.zeros((NH, 4, S), BFNP)
    for j in range(NH):
        s_hat = float(bf(np.float32(_SLOPES[half * NH + j])))
        qext[j, 0] = bf(np.full(S, s_hat))
        qext[j, 1] = bf(np.full(S, 256.0 * s_hat))
        qext[j, 2] = bf(-(_RIDX % 256))
        qext[j, 3] = bf(-(_RIDX // 256))
        kext[j, 0] = bf(_RIDX % 256)
        kext[j, 1] = bf(_RIDX // 256)
        kext[j, 2] = bf(np.full(S, s_hat))
        kext[j, 3] = bf(np.full(S, 256.0 * s_hat))

    out = {
        "x": f32(inp["x"][b]),
        "x_my": f32(inp["x"][b][half * ROWS_OUT:(half + 1) * ROWS_OUT]),
        "wq8": f8((wq[hsl] * g1[None, :]).T * WS),
        "wk8": f8((wk[hsl] * g1[None, :]).T * WS),
        "wv8": f8((wv[hsl] * g1[None, :]).T * WS),
        "woT": bf(wo[:, hsl].T),
        "w18": f8((w1 * g2[None, :]).T * WS),
        "w38": f8((w3 * g2[None, :]).T * WS),
        "w2T": bf(w2.T),
        "qkvbias": qkvbias,
        "b13": b13,
        "qext": qext,
        "kext": kext,
    }
    return {k: np.ascontiguousarray(v) for k, v in out.items()}


_COMPILED_NC = None
LAST_EXEC_NS = None


def _get_nc():
    global _COMPILED_NC
    if _COMPILED_NC is None:
        nc = bacc.Bacc("TRN2", target_bir_lowering=False, debug=False,
                       num_devices=N_CORES)
        _build_kernel(nc)
        nc.compile()
        _COMPILED_NC = nc
    return _COMPILED_NC


def kernel(**inputs):
    global LAST_EXEC_NS
    nc = _get_nc()
    in_maps = [_prep_core_inputs(inputs, c) for c in range(N_CORES)]
    trace = os.environ.get("KERNEL_TRACE", "0") == "1"
    res = run_bass_kernel_spmd(nc, in_maps, list(range(N_CORES)), trace=trace)
    LAST_EXEC_NS = res.exec_time_ns
    B = inputs["x"].shape[0]
    yout = np.zeros((B, S, D), np.float32)
    for core in range(N_CORES):
        b, half = core // 2, core % 2
        yout[b, half * ROWS_OUT:(half + 1) * ROWS_OUT] = res.results[core]["y"]
    return yout


# revision 7
# speedup vs baseline: 1.2506x; 1.2382x over previous
"""Trainium2 Bass kernel for nn_AttentionBlock (pre-LN MHA with ALiBi +
pre-LN SwiGLU FFN), distributed over 8 NeuronCores.

Sharding: core = (batch, head-half). Each core computes LN1 + QKV +
attention for 8 of 16 heads over its batch's full 2048 rows, a partial
O-projection, then quarter-chunked pairwise ReduceScatters combine the
partials (overlapped with the attention tail) and each core finishes
LN2 + SwiGLU FFN for its 1024 rows.

Precision: fp8e4m3 DoubleRow matmuls for QKV, AV, and FFN mm1/mm3
(weights pre-scaled x32 to avoid fp8 subnormals); bf16 for scores
(ALiBi folded into 4 augmentation contraction rows), O-proj and FFN
mm2. Attention is processed span-by-span (512 q rows), fully
interleaved with LN1/QKV production, with the AV matmul transposed
(out = v^T @ at) so the softmax denominator is a free 65th output row
and no o-transposes are needed.
"""

import os
from contextlib import ExitStack

import numpy as np
import ml_dtypes

import concourse.bass as bass
import concourse.bacc as bacc
import concourse.mybir as mybir
import concourse.tile as tile
from concourse.masks import make_identity
from concourse.bass_utils import run_bass_kernel_spmd

BFNP = ml_dtypes.bfloat16
F8NP = ml_dtypes.float8_e4m3
bf = lambda a: np.asarray(a).astype(BFNP)
f8 = lambda a: np.asarray(a, np.float32).astype(F8NP)
f32 = lambda a: np.asarray(a, np.float32)

FP32 = mybir.dt.float32
BF16 = mybir.dt.bfloat16
FP8 = mybir.dt.float8e4
AF = mybir.ActivationFunctionType
ALU = mybir.AluOpType
DR = mybir.MatmulPerfMode.DoubleRow

S = 2048
N_CORES = 8
D = 1024          # model dim
NH = 8            # heads per core
DH = 64           # head dim
DV = NH * DH      # 512, per-core qkv dim
ROWS_OUT = 1024   # rows per core after ReduceScatter
NKT = S // 128    # 16 k-tiles
NSP = 4           # spans of 512 q rows
WS = 32.0         # fp8 weight pre-scale
PAIRS = [[0, 1], [2, 3], [4, 5], [6, 7]]


def _build_kernel(nc):
    x = nc.dram_tensor("x", [S, D], FP32, kind="ExternalInput").ap()
    x_my = nc.dram_tensor("x_my", [ROWS_OUT, D], FP32, kind="ExternalInput").ap()
    wq8 = nc.dram_tensor("wq8", [D, DV], FP8, kind="ExternalInput").ap()
    wk8 = nc.dram_tensor("wk8", [D, DV], FP8, kind="ExternalInput").ap()
    wv8 = nc.dram_tensor("wv8", [D, DV], FP8, kind="ExternalInput").ap()
    woT = nc.dram_tensor("woT", [DV, D], BF16, kind="ExternalInput").ap()
    w18 = nc.dram_tensor("w18", [D, D], FP8, kind="ExternalInput").ap()
    w38 = nc.dram_tensor("w38", [D, D], FP8, kind="ExternalInput").ap()
    w2T = nc.dram_tensor("w2T", [D, D], BF16, kind="ExternalInput").ap()
    qkvbias = nc.dram_tensor("qkvbias", [3, DV], FP32, kind="ExternalInput").ap()
    b13 = nc.dram_tensor("b13", [2, D], FP32, kind="ExternalInput").ap()
    qext = nc.dram_tensor("qext", [NH, 4, S], BF16, kind="ExternalInput").ap()
    kext = nc.dram_tensor("kext", [NH, 4, S], BF16, kind="ExternalInput").ap()

    y = nc.dram_tensor("y", [ROWS_OUT, D], FP32, kind="ExternalOutput").ap()

    with tile.TileContext(nc) as tc, ExitStack() as ctx:
        singles = ctx.enter_context(tc.tile_pool(name="singles", bufs=1))
        dram = ctx.enter_context(tc.tile_pool(name="dram", bufs=1, space="DRAM"))

        ident = singles.tile([128, 128], BF16)
        make_identity(nc, ident)

        # per-partition biases for Q/K evac ([128, 4] : dout-in-chunk x chunk)
        qb_sb = singles.tile([128, 4], FP32)
        kb_sb = singles.tile([128, 4], FP32)
        nc.sync.dma_start(out=qb_sb, in_=qkvbias[0].rearrange("(c p) -> p c", p=128))
        nc.sync.dma_start(out=kb_sb, in_=qkvbias[1].rearrange("(c p) -> p c", p=128))
        # V bias broadcast along partitions [128, 512]
        vb_sb = singles.tile([128, DV], FP32)
        nc.sync.dma_start(
            out=vb_sb,
            in_=bass.AP(tensor=qkvbias.tensor, offset=2 * DV, ap=[[0, 128], [1, DV]]),
        )
        b1_sb = singles.tile([128, 8], FP32)
        b3_sb = singles.tile([128, 8], FP32)
        nc.sync.dma_start(out=b1_sb, in_=b13[0].rearrange("(c p) -> p c", p=128))
        nc.sync.dma_start(out=b3_sb, in_=b13[1].rearrange("(c p) -> p c", p=128))

        # x2 residual rows (mine) - filled with x_my early, += RS result later
        p_x2_cm = tc.tile_pool(name="p_x2", bufs=1)
        p_x2 = p_x2_cm.__enter__()
        x2_sb = p_x2.tile([128, 8, D], FP32)

        # persistent tensors for phases 1-4
        p_big_cm = tc.tile_pool(name="p_big", bufs=1)
        p_big = p_big_cm.__enter__()
        hT = p_big.tile([128, 8, S], FP8)        # LN1-out transposed, c x rows
        qaug = [p_big.tile([68, S], BF16, name=f"qaug{h}", tag=f"qaug{h}")
                for h in range(NH)]
        kaug = [p_big.tile([68, S], BF16, name=f"kaug{h}", tag=f"kaug{h}")
                for h in range(NH)]
        vsb = p_big.tile([128, NKT, NH, 72], FP8)  # v rows + ones col, pad 72
        oT = p_big.tile([128, 4, S], BF16)         # normalized o, feature-major
        wo_sb = p_big.tile([128, 4, D], BF16)

        # DRAM comm buffers: quarter q rows = [1024k + 256q, +256) for rank k
        ccin = [dram.tile([2, 256, D], BF16, name=f"ccin{q}") for q in range(4)]
        ccout = [dram.tile([256, D], BF16, name=f"ccout{q}") for q in range(4)]

        # ---------------- phases 1-3 interleaved by span ----------------
        with tc.tile_pool(name="qkvw", bufs=1) as qkvw, \
             tc.tile_pool(name="xgp", bufs=2) as xgp, \
             tc.tile_pool(name="ln1", bufs=3) as ln1p, \
             tc.tile_pool(name="attp", bufs=3) as attp, \
             tc.tile_pool(name="attp2", bufs=2) as attp2, \
             tc.tile_pool(name="atd", bufs=1) as atdp, \
             tc.tile_pool(name="oprs", bufs=2) as oprs, \
             tc.tile_pool(name="pwork", bufs=2, space="PSUM") as pwork, \
             tc.tile_pool(name="pacc", bufs=3, space="PSUM") as pacc, \
             tc.tile_pool(name="ptr", bufs=1, space="PSUM") as ptrp:
            wq_sb = qkvw.tile([128, 8, DV], FP8, tag="wq")
            wk_sb = qkvw.tile([128, 8, DV], FP8, tag="wk")
            wv_sb = qkvw.tile([128, 8, DV], FP8, tag="wv")
            nc.gpsimd.dma_start(out=wq_sb, in_=wq8.rearrange("(c p) n -> p c n", p=128))
            nc.gpsimd.dma_start(out=wk_sb, in_=wk8.rearrange("(c p) n -> p c n", p=128))
            nc.gpsimd.dma_start(out=wv_sb, in_=wv8.rearrange("(c p) n -> p c n", p=128))
            nc.gpsimd.dma_start(out=wo_sb, in_=woT.rearrange("(c p) n -> p c n", p=128))
            nc.gpsimd.memset(vsb, 1.0)
            for h in range(NH):
                nc.gpsimd.dma_start(out=qaug[h][64:68, :], in_=qext[h])
                nc.gpsimd.dma_start(out=kaug[h][64:68, :], in_=kext[h])
            # dedicated diagonal at tiles (pre-zeroed masked regions), by
            # in-span diag pair position p and head parity
            at_diag = [[atdp.tile([128, 2, 512], FP8, name=f"atd{p}_{par}",
                                  tag=f"atd{p}_{par}")
                        for par in range(2)] for p in range(2)]
            for p in range(2):
                for par in range(2):
                    nc.vector.memset(at_diag[p][par], 0.0)

            for g in range(NSP):
                if g == 3:
                    # x_my prefetch into x2 residual buffer (needed in P5-6)
                    for rt in range(8):
                        nc.sync.dma_start(
                            out=x2_sb[:, rt, :],
                            in_=x_my[rt * 128:(rt + 1) * 128, :])
                # ---- LN1 + transpose for row group g (512 rows) ----
                xgs = []
                for gh in range(2):
                    xg = xgp.tile([128, 2, D], FP32, tag="xg", name=f"xg{g}_{gh}")
                    nc.sync.dma_start(
                        out=xg,
                        in_=x[g * 512 + gh * 256:g * 512 + gh * 256 + 256, :]
                            .rearrange("(a p) d -> p a d", p=128))
                    xgs.append(xg)
                for rt in range(4 * g, 4 * g + 4):
                    xt = xgs[(rt % 4) // 2][:, rt % 2, :]
                    stats = ln1p.tile([128, 2, 6], FP32, tag="stats")
                    xr = xt.rearrange("p (s f) -> p s f", s=2)
                    nc.vector.bn_stats(out=stats[:, 0, :], in_=xr[:, 0, :])
                    nc.vector.bn_stats(out=stats[:, 1, :], in_=xr[:, 1, :])
                    mv = ln1p.tile([128, 2], FP32, tag="mv")
                    nc.vector.bn_aggr(out=mv, in_=stats)
                    # rstd = rsqrt(var + eps) via 2 Newton iterations from 1.0
                    ve = ln1p.tile([128, 3], FP32, tag="ve")
                    nc.vector.tensor_scalar(
                        out=ve[:, 0:1], in0=mv[:, 1:2], scalar1=1.0, scalar2=1e-5,
                        op0=ALU.mult, op1=ALU.add)
                    nc.vector.tensor_scalar(
                        out=ve[:, 1:2], in0=ve[:, 0:1], scalar1=-0.5, scalar2=1.5,
                        op0=ALU.mult, op1=ALU.add)
                    nc.vector.tensor_mul(ve[:, 2:3], ve[:, 1:2], ve[:, 1:2])
                    nc.vector.tensor_mul(ve[:, 2:3], ve[:, 2:3], ve[:, 0:1])
                    nc.vector.tensor_scalar(
                        out=ve[:, 2:3], in0=ve[:, 2:3], scalar1=-0.5, scalar2=1.5,
                        op0=ALU.mult, op1=ALU.add)
                    rstd = ln1p.tile([128, 1], FP32, tag="rstd")
                    nc.vector.tensor_mul(rstd, ve[:, 1:2], ve[:, 2:3])
                    hrow = ln1p.tile([128, D], BF16, tag="hrow")
                    nc.vector.tensor_scalar(
                        out=hrow, in0=xt, scalar1=mv[:, 0:1], scalar2=rstd,
                        op0=ALU.subtract, op1=ALU.mult)
                    pt = ptrp.tile([128, 8, 128], BF16, tag="pt")
                    for c in range(8):
                        nc.tensor.transpose(pt[:, c, :], hrow[:, c * 128:(c + 1) * 128],
                                            ident)
                    nc.vector.tensor_copy(hT[:, :, rt * 128:(rt + 1) * 128], pt)

                # ---- QKV for row group g (fp8 DoubleRow), interleaved
                # per m-chunk so attention heads can start early ----
                for m in range(4):
                    for (w_sb, aug, bias, scale) in (
                        (wq_sb, qaug, qb_sb, 0.125 / WS),
                        (wk_sb, kaug, kb_sb, 1.0 / WS),
                    ):
                        ps = pwork.tile([128, 2, 512], FP32, tag="w")
                        for c in range(4):
                            nc.tensor.matmul(
                                ps[:, 0, :],
                                lhsT=w_sb[:, 2 * c:2 * c + 2, m * 128:(m + 1) * 128],
                                rhs=hT[:, 2 * c:2 * c + 2, g * 512:(g + 1) * 512],
                                start=(c == 0), stop=(c == 3), perf_mode=DR)
                        for sub in range(2):
                            nc.vector.tensor_scalar(
                                out=aug[2 * m + sub][0:64, g * 512:(g + 1) * 512],
                                in0=ps[sub * 64:sub * 64 + 64, 0, :],
                                scalar1=scale,
                                scalar2=bias[sub * 64:sub * 64 + 64, m:m + 1],
                                op0=ALU.mult, op1=ALU.add)
                    kt = 4 * g + m
                    ps = pwork.tile([128, 2, 512], FP32, tag="w")
                    for c in range(4):
                        nc.tensor.matmul(
                            ps[:, 0, :],
                            lhsT=hT[:, 2 * c:2 * c + 2, kt * 128:(kt + 1) * 128],
                            rhs=wv_sb[:, 2 * c:2 * c + 2, :],
                            start=(c == 0), stop=(c == 3), perf_mode=DR)
                    nc.vector.scalar_tensor_tensor(
                        out=vsb[:, kt, :, 0:64],
                        in0=ps[:, 0, :].rearrange("p (h e) -> p h e", h=NH),
                        scalar=1.0 / WS,
                        in1=vb_sb.rearrange("p (h e) -> p h e", h=NH),
                        op0=ALU.mult, op1=ALU.add)

                # ---- attention span g for all heads ----
                npair = 2 * (g + 1)
                pend_mult = []     # (h, oacc, rb) awaiting normalize-evac
                pend_av = []       # (i, h, oacc, at) awaiting AV matmul

                def emit_avs(pend_av=pend_av, pend_mult=pend_mult,
                             npair=npair, g=g):
                    # emit deferred AV matmuls (so the tensor queue never
                    # sits waiting on an exp); on a head's last AV also
                    # emit its (deferred) normalize-evac chain
                    while pend_av:
                        i2, h2, oacc2, at2 = pend_av.pop(0)
                        nc.tensor.matmul(
                            oacc2[0:65, :],
                            lhsT=vsb[:, 2 * i2:2 * i2 + 2, h2, 0:65],
                            rhs=at2,
                            start=(i2 == 0), stop=(i2 == npair - 1),
                            perf_mode=DR)
                        if i2 == npair - 1:
                            den = attp2.tile([1, 512], FP32, tag="den")
                            nc.vector.tensor_copy(den, oacc2[64:65, :])
                            rec = attp2.tile([1, 512], FP32, tag="rec")
                            nc.vector.reciprocal_approx_fast(out=rec, in_=den)
                            rb = attp2.tile([64, 512], FP32, tag="rb")
                            nc.gpsimd.partition_broadcast(rb, rec, channels=64)
                            pend_mult.append((h2, oacc2, rb))
                            if len(pend_mult) == 2:
                                hh, oo, rr = pend_mult.pop(0)
                                nc.vector.tensor_mul(
                                    oT[(hh % 2) * 64:(hh % 2) * 64 + 64,
                                       hh // 2, g * 512:(g + 1) * 512],
                                    oo[0:64, :], rr)

                for h in range(NH):
                    oacc = pacc.tile([128, 512], FP32, tag="acc",
                                     name=f"oa{g}_{h}")
                    for i in range(npair):
                        diag = i >= 2 * g
                        sc = pwork.tile([128, 2, 512], FP32, tag="w",
                                        name=f"sc{g}_{h}_{i}")
                        if not diag:
                            for j in range(2):
                                kt = 2 * i + j
                                nc.tensor.matmul(
                                    sc[:, j, :],
                                    lhsT=kaug[h][:, kt * 128:(kt + 1) * 128],
                                    rhs=qaug[h][:, g * 512:(g + 1) * 512],
                                    start=True, stop=True)
                            emit_avs()
                            at = attp.tile([128, 2, 512], FP8, tag="at",
                                           name=f"at{g}_{h}_{i}")
                            nc.scalar.activation(out=at, in_=sc, func=AF.Exp)
                        else:
                            p = i - 2 * g
                            at = at_diag[p][h % 2]
                            for j in range(2):
                                kt = 2 * i + j
                                jp = kt - 4 * g    # 0..3 within diag band
                                nc.tensor.matmul(
                                    sc[:, j, jp * 128:512],
                                    lhsT=kaug[h][:, kt * 128:(kt + 1) * 128],
                                    rhs=qaug[h][:, g * 512 + jp * 128:(g + 1) * 512],
                                    start=True, stop=True)
                                if j == 1:
                                    emit_avs()
                                nc.scalar.activation(
                                    out=at[:, j, jp * 128:512],
                                    in_=sc[:, j, jp * 128:512], func=AF.Exp)
                                nc.gpsimd.affine_select(
                                    out=at[:, j, jp * 128:jp * 128 + 128],
                                    in_=at[:, j, jp * 128:jp * 128 + 128],
                                    compare_op=ALU.is_ge, fill=0.0,
                                    base=0, channel_multiplier=-1,
                                    pattern=[[1, 128]])
                        pend_av.append((i, h, oacc, at))
                emit_avs()
                for hh, oo, rr in pend_mult:
                    nc.vector.tensor_mul(
                        oT[(hh % 2) * 64:(hh % 2) * 64 + 64, hh // 2,
                           g * 512:(g + 1) * 512],
                        oo[0:64, :], rr)
                    # normalize + evac into oT (deferred by one head so
                    # the vector queue never waits on the gpsimd broadcast)

                # ---- O-proj partials for span g + ccin DMA ----
                for rt4 in range(4):
                    rowslc = slice(g * 512 + rt4 * 128, g * 512 + rt4 * 128 + 128)
                    ob = oprs.tile([128, D], BF16, tag="ob")
                    for n in range(2):
                        ps = pwork.tile([128, 2, 512], FP32, tag="w")
                        for c in range(4):
                            nc.tensor.matmul(
                                ps[:, 0, :],
                                lhsT=oT[:, c, rowslc],
                                rhs=wo_sb[:, c, n * 512:(n + 1) * 512],
                                start=(c == 0), stop=(c == 3))
                        nc.scalar.copy(out=ob[:, n * 512:(n + 1) * 512],
                                       in_=ps[:, 0, :])
                    qq = 2 * (g % 2) + rt4 // 2
                    k = g // 2
                    u = rt4 % 2
                    nc.sync.dma_start(
                        out=ccin[qq][k][u * 128:u * 128 + 128, :], in_=ob)
                # fire quarter ReduceScatters when both spans contributed
                if g == 2:
                    for qq in range(2):
                        nc.gpsimd.collective_compute(
                            "ReduceScatter", ALU.add,
                            ins=[ccin[qq].opt()], outs=[ccout[qq].opt()],
                            replica_groups=PAIRS)
                if g == 3:
                    for qq in range(2, 4):
                        nc.gpsimd.collective_compute(
                            "ReduceScatter", ALU.add,
                            ins=[ccin[qq].opt()], outs=[ccout[qq].opt()],
                            replica_groups=PAIRS)

        p_big_cm.__exit__(None, None, None)

        # ---------------- phases 5-6 per quarter ----------------
        with tc.tile_pool(name="ffnw", bufs=1) as ffnw, \
             tc.tile_pool(name="ln2", bufs=3) as ln2p, \
             tc.tile_pool(name="ffn2", bufs=2) as ffn2, \
             tc.tile_pool(name="pacc6", bufs=3, space="PSUM") as pacc6, \
             tc.tile_pool(name="ptr5", bufs=1, space="PSUM") as ptr5:
            w1_sb = ffnw.tile([128, 8, D], FP8, tag="w1")
            w3_sb = ffnw.tile([128, 8, D], FP8, tag="w3")
            w2_sb = ffnw.tile([128, 8, D], BF16, tag="w2")
            nc.gpsimd.dma_start(out=w1_sb, in_=w18.rearrange("(c p) n -> p c n", p=128))
            nc.gpsimd.dma_start(out=w3_sb, in_=w38.rearrange("(c p) n -> p c n", p=128))
            nc.gpsimd.dma_start(out=w2_sb, in_=w2T.rearrange("(c p) n -> p c n", p=128))
            h2T = ffnw.tile([128, 8, ROWS_OUT], FP8, tag="h2T")

            for qq in range(4):
                rs = ln2p.tile([128, 2, D], BF16, tag="rs")
                nc.sync.dma_start(
                    out=rs, in_=ccout[qq].rearrange("(a p) d -> p a d", p=128))
                for u in range(2):
                    rt = 2 * qq + u
                    nc.vector.tensor_add(x2_sb[:, rt, :], rs[:, u, :],
                                         x2_sb[:, rt, :])
                    stats = ln2p.tile([128, 2, 6], FP32, tag="stats")
                    x2r = x2_sb[:, rt, :].rearrange("p (s f) -> p s f", s=2)
                    nc.vector.bn_stats(out=stats[:, 0, :], in_=x2r[:, 0, :])
                    nc.vector.bn_stats(out=stats[:, 1, :], in_=x2r[:, 1, :])
                    mv = ln2p.tile([128, 2], FP32, tag="mv")
                    nc.vector.bn_aggr(out=mv, in_=stats)
                    ve = ln2p.tile([128, 3], FP32, tag="ve")
                    nc.vector.tensor_scalar(
                        out=ve[:, 0:1], in0=mv[:, 1:2], scalar1=1.0, scalar2=1e-5,
                        op0=ALU.mult, op1=ALU.add)
                    nc.vector.tensor_scalar(
                        out=ve[:, 1:2], in0=ve[:, 0:1], scalar1=-0.5, scalar2=1.5,
                        op0=ALU.mult, op1=ALU.add)
                    nc.vector.tensor_mul(ve[:, 2:3], ve[:, 1:2], ve[:, 1:2])
                    nc.vector.tensor_mul(ve[:, 2:3], ve[:, 2:3], ve[:, 0:1])
                    nc.vector.tensor_scalar(
                        out=ve[:, 2:3], in0=ve[:, 2:3], scalar1=-0.5, scalar2=1.5,
                        op0=ALU.mult, op1=ALU.add)
                    rstd = ln2p.tile([128, 1], FP32, tag="rstd")
                    nc.vector.tensor_mul(rstd, ve[:, 1:2], ve[:, 2:3])
                    hrow = ln2p.tile([128, D], BF16, tag="hrow")
                    nc.vector.tensor_scalar(
                        out=hrow, in0=x2_sb[:, rt, :], scalar1=mv[:, 0:1],
                        scalar2=rstd, op0=ALU.subtract, op1=ALU.mult)
                    pt = ptr5.tile([128, 8, 128], BF16, tag="pt")
                    for c in range(8):
                        nc.tensor.transpose(pt[:, c, :], hrow[:, c * 128:(c + 1) * 128],
                                            ident)
                    nc.vector.tensor_copy(h2T[:, :, rt * 128:(rt + 1) * 128], pt)

                # FFN for this quarter (256 rows)
                cols = slice(qq * 256, qq * 256 + 256)
                gs = ffn2.tile([128, 8, 256], BF16, tag="gs")
                for f in range(8):
                    ps = pacc6.tile([128, 512], FP32, tag="acc6")
                    for c in range(4):
                        nc.tensor.matmul(
                            ps[:, 0:256],
                            lhsT=w1_sb[:, 2 * c:2 * c + 2, f * 128:(f + 1) * 128],
                            rhs=h2T[:, 2 * c:2 * c + 2, cols],
                            start=(c == 0), stop=(c == 3), perf_mode=DR)
                    us = ffn2.tile([128, 256], BF16, tag="us")
                    nc.scalar.activation(
                        out=us, in_=ps[:, 0:256], func=AF.Silu,
                        bias=b1_sb[:, f:f + 1], scale=1.0 / WS)
                    ps2 = pacc6.tile([128, 512], FP32, tag="acc6")
                    for c in range(4):
                        nc.tensor.matmul(
                            ps2[:, 0:256],
                            lhsT=w3_sb[:, 2 * c:2 * c + 2, f * 128:(f + 1) * 128],
                            rhs=h2T[:, 2 * c:2 * c + 2, cols],
                            start=(c == 0), stop=(c == 3), perf_mode=DR)
                    ts = ffn2.tile([128, 256], BF16, tag="ts")
                    nc.vector.tensor_scalar(
                        out=ts, in0=ps2[:, 0:256], scalar1=1.0 / WS,
                        scalar2=b3_sb[:, f:f + 1], op0=ALU.mult, op1=ALU.add)
                    nc.vector.tensor_mul(gs[:, f, :], us, ts)
                for u in range(2):
                    rt = 2 * qq + u
                    ysb = ffn2.tile([128, D], FP32, tag="ysb")
                    for n in range(2):
                        ps = pacc6.tile([128, 512], FP32, tag="acc6")
                        for f in range(8):
                            nc.tensor.matmul(
                                ps,
                                lhsT=gs[:, f, u * 128:u * 128 + 128],
                                rhs=w2_sb[:, f, n * 512:(n + 1) * 512],
                                start=(f == 0), stop=(f == 7))
                        nc.vector.tensor_add(
                            ysb[:, n * 512:(n + 1) * 512], ps,
                            x2_sb[:, rt, n * 512:(n + 1) * 512])
                    nc.sync.dma_start(out=y[rt * 128:(rt + 1) * 128, :], in_=ysb)

        p_x2_cm.__exit__(None, None, None)
    return nc


H = 16
_SLOPES = (2.0 ** (-8.0 / H)) ** np.arange(1, H + 1)
_RIDX = np.arange(S, dtype=np.float64)


def _prep_core_inputs(inp, core):
    b, half = core // 2, core % 2
    hsl = slice(half * DV, (half + 1) * DV)
    g1, b1 = f32(inp["ln1_g"]), f32(inp["ln1_b"])
    g2, b2 = f32(inp["ln2_g"]), f32(inp["ln2_b"])
    wq, wk, wv, wo = (f32(inp[k]) for k in ("wq", "wk", "wv", "wo"))
    w1, w2, w3 = (f32(inp[k]) for k in ("w1", "w2", "w3"))

    qkvbias = np.stack([
        (wq[hsl] @ b1) / 8.0,
        wk[hsl] @ b1,
        wv[hsl] @ b1,
    ]).astype(np.float32)
    b13 = np.stack([w1 @ b2, w3 @ b2]).astype(np.float32)

    qext = np.zeros((NH, 4, S), BFNP)
    kext = np.zeros((NH, 4, S), BFNP)
    for j in range(NH):
        s_hat = float(bf(np.float32(_SLOPES[half * NH + j])))
        qext[j, 0] = bf(np.full(S, s_hat))
        qext[j, 1] = bf(np.full(S, 256.0 * s_hat))
        qext[j, 2] = bf(-(_RIDX % 256))
        qext[j, 3] = bf(-(_RIDX // 256))
        kext[j, 0] = bf(_RIDX % 256)
        kext[j, 1] = bf(_RIDX // 256)
        kext[j, 2] = bf(np.full(S, s_hat))
        kext[j, 3] = bf(np.full(S, 256.0 * s_hat))

    out = {
        "x": f32(inp["x"][b]),
        "x_my": f32(inp["x"][b][half * ROWS_OUT:(half + 1) * ROWS_OUT]),
        "wq8": f8((wq[hsl] * g1[None, :]).T * WS),
        "wk8": f8((wk[hsl] * g1[None, :]).T * WS),
        "wv8": f8((wv[hsl] * g1[None, :]).T * WS),
        "woT": bf(wo[:, hsl].T),
        "w18": f8((w1 * g2[None, :]).T * WS),
        "w38": f8((w3 * g2[None, :]).T * WS),
        "w2T": bf(w2.T),
        "qkvbias": qkvbias,
        "b13": b13,
        "qext": qext,
        "kext": kext,
    }
    return {k: np.ascontiguousarray(v) for k, v in out.items()}


_COMPILED_NC = None
LAST_EXEC_NS = None


def _get_nc():
    global _COMPILED_NC
    if _COMPILED_NC is None:
        nc = bacc.Bacc("TRN2", target_bir_lowering=False, debug=False,
                       num_devices=N_CORES)
        _build_kernel(nc)
        nc.compile()
        _COMPILED_NC = nc
    return _COMPILED_NC


def kernel(**inputs):
    global LAST_EXEC_NS
    nc = _get_nc()
    in_maps = [_prep_core_inputs(inputs, c) for c in range(N_CORES)]
    trace = os.environ.get("KERNEL_TRACE", "0") == "1"
    res = run_bass_kernel_spmd(nc, in_maps, list(range(N_CORES)), trace=trace)
    LAST_EXEC_NS = res.exec_time_ns
    B = inputs["x"].shape[0]
    yout = np.zeros((B, S, D), np.float32)
    for core in range(N_CORES):
        b, half = core // 2, core % 2
        yout[b, half * ROWS_OUT:(half + 1) * ROWS_OUT] = res.results[core]["y"]
    return yout
